# revision 17
# baseline (speedup 1.0000x reference)
"""GCN 3-layer (EnhancedLinkPredictor) on 8 Trainium2 NeuronCores — v4.

Strategy (1D destination sharding, aggregate-then-matmul, PE segment-sum):
  Nodes are snake-assigned to cores by in-degree; each core's 12544 local
  nodes are bin-packed into 98 tiles of 128 under rotating per-(tile,bucket)
  slot caps (640/512).  Pad slots carry dummy index 0 (valid row; its
  one-hot column is zero) so gather runs have no embedded negative indices.

  v4 vs v2: the idx stream is laid out bucket-major ((bucket, tile) runs)
  and gathered in uniform ring-sized calls of 8 windows (1024 descriptors,
  the SWDGE ring capacity), decoupled from tile boundaries — ~210 gather
  instructions per layer instead of 392.  GpSimd desc-gen (~2.1ns/desc,
  ucode-batched) was the serializing resource (99-100% engine occupancy in
  every layer of the v2 trace).  Gather call buffers shrink to one call
  ([128,8,128] f16) with 4 pool bufs per bucket.  PSUM->SBUF tail copies
  move from DVE to the Scalar engine to unload the #2 engine.

  Per layer: fp16 table [100352,128] is AllGathered; per call a
  non-transpose dma_gather (4 SWDGE queues, queue=bucket) pulls src rows
  node-major into SBUF windows of 128 slots; PE accumulates
  Zt[feat,dst] += gathered_win^T @ onehot(win) in PSUM, where the one-hot
  [slot,dstpos] matrices are DVE-generated from a dstpos stream
  (is_equal against an iota row); self-loops use the SBUF-resident
  previous-layer tile against an fp16 identity.  Tails apply the dst-side
  dinv scaling, bias, relu and the layer matmuls.

  Tile's DMASW completion lanes are made queue-aware (lane = SWDGE queue)
  so multi-queue gathers keep in-order semaphore semantics.
"""

import numpy as np

N = 100000
E = 1600000
F = 128              # table feature width
HID = 64
OUT_C = 64
NCORES = 8
USED = 12544         # nodes per core (98 tiles of 128)
SHARD = 12544        # storage rows per core (no pad rows)
NTILES = USED // 128  # 98
CAP_HI, CAP_LO = 640, 512
TILE_SLOTS = CAP_HI + 3 * CAP_LO          # 2176
L_STREAM = NTILES * TILE_SLOTS            # 213248
NWIN = L_STREAM // 128                    # 1666
CALL_W = 8           # windows per gather call (1024 descs = ring size)
NQ = 4

_CACHE = {}
LAST_RESULT = None


def _caps():
    caps = np.full((NTILES, 4), CAP_LO, np.int64)
    for t in range(NTILES):
        caps[t, t % 4] = CAP_HI
    return caps


# ----------------------------------------------------------------------------
# host-side graph preprocessing
# ----------------------------------------------------------------------------
def _pack_tiles(v, caps):
    """Pack len(v) items with 4-dim weights v into NTILES bins of <=128 items
    with per-bin capacity caps[t]. Returns tile index per item."""
    nit = len(v)
    order = np.argsort(-v.sum(axis=1), kind="stable")
    rem = caps.astype(np.int64).copy()
    cnt = np.full(NTILES, 128, np.int64)
    tile_of = np.empty(nit, np.int64)
    for i in order:
        vi = v[i]
        feas = (cnt > 0) & np.all(rem >= vi[None, :], axis=1)
        if not feas.any():
            feas = cnt > 0
        slack = (rem - vi[None, :]).min(axis=1).astype(np.float64)
        slack += 1e-3 * rem.sum(axis=1)
        slack[~feas] = -1e18
        t = int(np.argmax(slack))
        tile_of[i] = t
        rem[t] -= vi
        cnt[t] -= 1
    return tile_of, rem.min() >= 0


def _preprocess(edge_index):
    src = edge_index[0].astype(np.int64)
    dst = edge_index[1].astype(np.int64)
    deg_in = np.bincount(dst, minlength=N)

    # snake assignment by in-degree: balances per-core edge counts
    order = np.argsort(-deg_in, kind="stable")
    rank = np.empty(N, np.int64)
    rank[order] = np.arange(N)
    chunk, pos = rank // NCORES, rank % NCORES
    core = np.where(chunk % 2 == 0, pos, NCORES - 1 - pos)

    # per-node bucket in-degree vectors (bucket = src core-pair)
    nb = np.zeros((N, 4), np.int64)
    np.add.at(nb, (dst, core[src] // 2), 1)

    caps = _caps()
    local = np.full(N, -1, np.int64)
    for c in range(NCORES):
        ids = np.where(core == c)[0]
        tile_of, ok = _pack_tiles(nb[ids], caps)
        assert ok, f"tile packing overflow on core {c}"
        loc = np.empty(len(ids), np.int64)
        for t in range(NTILES):
            sel = np.where(tile_of == t)[0]
            assert len(sel) <= 128
            loc[sel] = t * 128 + np.arange(len(sel))
        local[ids] = loc
    storage = core * SHARD + local

    # idx-stream offsets: (bucket, tile) order — per-bucket contiguous runs,
    # gathered in uniform CALL_W-window calls decoupled from tile boundaries
    base_b = np.zeros(5, np.int64)
    for b in range(4):
        base_b[b + 1] = base_b[b] + caps[:, b].sum()
    assert base_b[4] == L_STREAM
    off_tb = np.zeros((NTILES, 4), np.int64)
    for b in range(4):
        run = base_b[b]
        for t in range(NTILES):
            off_tb[t, b] = run
            run += caps[t, b]

    # dpos-stream offsets: (tile, bucket) order — matches per-tile one-hot
    # generation and the window order the matmul loop consumes
    off_dp = np.zeros((NTILES, 4), np.int64)
    run = 0
    for t in range(NTILES):
        for b in range(4):
            off_dp[t, b] = run
            run += caps[t, b]
    assert run == L_STREAM

    d_core = core[dst]
    d_local = local[dst]
    t_tile = d_local // 128
    dpos = d_local % 128
    bkt = core[src] // 2
    sidx = storage[src] - bkt * 2 * SHARD
    assert sidx.min() >= 0 and sidx.max() < 2 * SHARD <= 32768

    # rank of each edge within its (core,tile,bucket) group
    key = (d_core * NTILES + t_tile) * 4 + bkt
    order2 = np.argsort(key, kind="stable")
    ks = key[order2]
    starts = np.concatenate([[0], np.flatnonzero(np.diff(ks)) + 1])
    group_sizes = np.diff(np.concatenate([starts, [len(ks)]]))
    rank_sorted = np.arange(len(ks)) - np.repeat(starts, group_sizes)
    rnk = np.empty(len(ks), np.int64)
    rnk[order2] = rank_sorted
    assert (rnk < caps[t_tile, bkt]).all()

    # pads: idx 0 (valid dummy row, zeroed by one-hot), dpos 999
    idx_streams = np.zeros((NCORES, L_STREAM), np.int16)
    idx_streams[d_core, off_tb[t_tile, bkt] + rnk] = sidx.astype(np.int16)
    dpos_streams = np.full((NCORES, L_STREAM), 999.0, np.float32)
    dpos_streams[d_core, off_dp[t_tile, bkt] + rnk] = dpos.astype(np.float32)

    idx_wrapped = np.ascontiguousarray(
        np.tile(
            idx_streams.reshape(NCORES, L_STREAM // 16, 16).transpose(0, 2, 1),
            (1, 8, 1)))
    dposT = np.ascontiguousarray(
        dpos_streams.reshape(NCORES, NWIN, 128).transpose(0, 2, 1))

    deg = (deg_in + 1.0).astype(np.float32)
    degT = np.ones((NCORES, 128, NTILES), np.float32)
    degT[core, local % 128, local // 128] = deg

    geo = dict(caps=caps, off_tb=off_tb, base_b=base_b)
    return dict(core=core, local=local, degT=degT, idx=idx_wrapped,
                dposT=dposT, geo=geo)


# ----------------------------------------------------------------------------
# tile framework patch: queue-aware DMASW completion lanes
# ----------------------------------------------------------------------------
def _patch_tile_queue_lanes():
    import concourse.tile_sem_assignment as tsa
    from concourse import bass_isa
    import concourse.mybir as mybir
    if getattr(tsa.TileClockTick, "_qaware_patch", False):
        return
    orig = tsa.TileClockTick._assign_tick
    DMAInst = tsa.DMAInst

    def _assign_tick(self, inst):
        if (isinstance(inst, DMAInst)
                and not isinstance(inst, bass_isa.UserSyncedRemoteDMADescs)
                and inst.engine == mybir.EngineType.Pool):
            q = getattr(inst, "queue_num", 0) or 0
            self.next_sw_dma_idx = int(q) % self.swdge_sem_count
        return orig(self, inst)

    tsa.TileClockTick._assign_tick = _assign_tick
    tsa.TileClockTick._qaware_patch = True


# ----------------------------------------------------------------------------
# device program
# ----------------------------------------------------------------------------
def _build_program(geo):
    _patch_tile_queue_lanes()
    import concourse.bass as bass
    import concourse.mybir as mybir
    import concourse.tile as tile
    from concourse import bacc
    from concourse.bass import _add_dep_helper
    from concourse.library_config import mlp
    from concourse.masks import make_identity

    caps = geo["caps"]
    off_tb = geo["off_tb"]
    base_b = geo["base_b"]
    f32, f16, i16 = mybir.dt.float32, mybir.dt.float16, mybir.dt.int16
    EQ = mybir.AluOpType.is_equal
    MUL = mybir.AluOpType.mult
    ADD = mybir.AluOpType.add
    COPY = mybir.ActivationFunctionType.Copy

    # per-bucket window counts and cumulative window offset of each (t,b)
    nwin_b = [int(caps[:, b].sum()) // 128 for b in range(4)]
    gw0 = np.zeros((NTILES, 4), np.int64)   # first window of (t,b) in bucket b
    for b in range(4):
        run = 0
        for t in range(NTILES):
            gw0[t, b] = run
            run += int(caps[t, b]) // 128
        assert run == nwin_b[b]

    nc = bacc.Bacc("TRN2", target_bir_lowering=False, debug=False,
                   num_devices=NCORES, num_swdge_queues=NQ)
    x_pre = nc.dram_tensor("x_pre", [128, NTILES * F], f16,
                           kind="ExternalInput")
    tabA0 = nc.dram_tensor("tabA0", [NCORES * SHARD, F], f16,
                           kind="ExternalInput")
    degT = nc.dram_tensor("degT", [128, NTILES], f32, kind="ExternalInput")
    idxs = nc.dram_tensor("idxs", [128, L_STREAM // 16], i16,
                          kind="ExternalInput")
    dposTd = nc.dram_tensor("dposT", [128, NWIN], f32, kind="ExternalInput")
    iotad = nc.dram_tensor("iota16", [128, 128], f16, kind="ExternalInput")
    ident16d = nc.dram_tensor("ident16", [128, 128], f16,
                              kind="ExternalInput")
    W1 = nc.dram_tensor("W1", [128, HID], f32, kind="ExternalInput")
    W2 = nc.dram_tensor("W2", [HID, 128], f32, kind="ExternalInput")
    W3 = nc.dram_tensor("W3", [128, OUT_C], f32, kind="ExternalInput")
    b1b = nc.dram_tensor("b1b", [128, HID], f32, kind="ExternalInput")
    b2b = nc.dram_tensor("b2b", [128, 128], f32, kind="ExternalInput")
    b3b = nc.dram_tensor("b3b", [128, OUT_C], f32, kind="ExternalInput")
    out_sh = nc.dram_tensor("out_sh", [USED, OUT_C], f32,
                            kind="ExternalOutput")
    shard = nc.dram_tensor("shard", [SHARD, F], f16, kind="Internal")
    tabA = nc.dram_tensor("tabA", [NCORES * SHARD, F], f16, kind="Internal",
                          addr_space="Shared")
    tabB = nc.dram_tensor("tabB", [NCORES * SHARD, F], f16, kind="Internal",
                          addr_space="Shared")

    with tile.TileContext(nc) as tc:
        with tc.tile_pool(name="const", bufs=1) as cp, \
             tc.tile_pool(name="gbuf", bufs=9) as gp, \
             tc.tile_pool(name="ohbuf", bufs=3) as op_, \
             tc.tile_pool(name="zbuf", bufs=3) as zp, \
             tc.tile_pool(name="ebuf", bufs=3) as ep, \
             tc.tile_pool(name="psum", bufs=2, space="PSUM") as pp:
            nc.gpsimd.load_library(mlp)

            idxt = cp.tile([128, L_STREAM // 16], i16)
            nc.sync.dma_start(idxt[:], idxs[:])
            ident = cp.tile([128, 128], f32)
            make_identity(nc, ident[:])
            w1 = cp.tile([128, HID], f32)
            nc.sync.dma_start(w1[:], W1[:])
            w2 = cp.tile([HID, 128], f32)
            nc.sync.dma_start(w2[:], W2[:])
            w3 = cp.tile([128, OUT_C], f32)
            nc.sync.dma_start(w3[:], W3[:])
            bb1 = cp.tile([128, HID], f32)
            nc.sync.dma_start(bb1[:], b1b[:])
            bb2 = cp.tile([128, 128], f32)
            nc.sync.dma_start(bb2[:], b2b[:])
            bb3 = cp.tile([128, OUT_C], f32)
            nc.sync.dma_start(bb3[:], b3b[:])
            iota16 = cp.tile([128, 128], f16)
            nc.sync.dma_start(iota16[:], iotad[:])
            dposTs = ep.tile([128, NWIN], f32, tag="dps")
            nc.sync.dma_start(dposTs[:], dposTd[:])
            dposT16 = cp.tile([128, NWIN], f16)
            nc.vector.tensor_copy(dposT16[:], dposTs[:])

            ident16 = cp.tile([128, 128], f16)
            nc.sync.dma_start(ident16[:], ident16d[:])

            # dinv = sqrt(1/deg)
            degt = cp.tile([128, NTILES], f32)
            nc.sync.dma_start(degt[:], degT[:])
            rec = cp.tile([128, NTILES], f32)
            nc.vector.reciprocal(rec[:], degt[:])
            dinv = cp.tile([128, NTILES], f32)
            nc.scalar.activation(dinv[:], rec[:],
                                 mybir.ActivationFunctionType.Sqrt)

            # resident previous-layer tables (node-major, dinv-scaled)
            yresA = cp.tile([128, NTILES * F], f16)
            yresB = cp.tile([128, NTILES * F], f16)

            # prologue: host-prescaled table1 straight into yresA + shard
            sc = nc.enter_named_scope("prologue", False)
            nc.sync.dma_start(yresA[:], x_pre[:])
            # warm up the SWDGE gather path on all queues while idle
            for w in range(NQ):
                wt = ep.tile([128, 1, 128], f16, tag="warm")
                nc.gpsimd.dma_gather(
                    wt[:], tabA0[0:2 * SHARD, :],
                    idxt[:, 0:8], 128, 128, F,
                    transpose=False, queue_num=w)
            nc.leave_named_scope("prologue", sc[0], False)

            def allgather(dst_tab, writes_by_chunk):
                cc = nc.gpsimd.collective_compute(
                    "AllGather", mybir.AluOpType.bypass,
                    replica_groups=[list(range(NCORES))],
                    ins=[shard[:]], outs=[dst_tab[:]])
                for deps in writes_by_chunk:
                    for d in deps:
                        _add_dep_helper(cc.ins, d.ins, sync=True,
                                        reason="shard writes before AG")
                return (cc,)

            def aggregate_layer(table, yres_in, tail, cc):
                writes = [[], [], [], []]
                chunk_of = np.searchsorted(
                    [25, 50, 74], np.arange(NTILES), side="right")
                next_call = [0, 0, 0, 0]
                call_tiles = {}

                def emit_call(b):
                    j = next_call[b]
                    w0, w1_ = j * CALL_W, min((j + 1) * CALL_W, nwin_b[b])
                    ni = (w1_ - w0) * 128
                    off = int(base_b[b]) + w0 * 128
                    ct = gp.tile([128, CALL_W, F], f16, name=f"cb{b}",
                                 tag=f"cb{b}")
                    gi = nc.gpsimd.dma_gather(
                        ct[:, 0:w1_ - w0, :],
                        table[b * 2 * SHARD:(b + 1) * 2 * SHARD, :],
                        idxt[:, off // 16:(off + ni) // 16],
                        ni, ni, F, transpose=False, queue_num=b)
                    for c_ in cc:
                        _add_dep_helper(gi.ins, c_.ins, sync=True,
                                        reason="AG before gathers")
                    call_tiles[(b, j)] = ct
                    next_call[b] = j + 1

                NWT = TILE_SLOTS // 128
                for t in range(NTILES):
                    # make sure calls covering this tile's windows exist
                    for b in range(4):
                        gend = int(gw0[t, b]) + int(caps[t, b]) // 128
                        while next_call[b] * CALL_W < gend:
                            emit_call(b)
                    psZ = pp.tile([128, 128], f32, tag="Z")
                    nc.tensor.matmul(psZ[:],
                                     lhsT=yres_in[:, t * F:(t + 1) * F],
                                     rhs=ident16[:],
                                     start=True, stop=False)
                    oh = op_.tile([128, NWT, 128], f16, tag="oh")
                    dpos3 = dposT16[:, t * NWT:(t + 1) * NWT].rearrange(
                        "p (a o) -> p a o", o=1)
                    iota3 = iota16[:].rearrange("p (a j) -> p a j", a=1)
                    i_b, d_b = bass.broadcast_tensor_aps(iota3, dpos3)
                    nc.vector.tensor_tensor(oh[:], i_b, d_b, EQ)
                    wk = 0
                    for b in range(4):
                        nw_ = int(caps[t, b]) // 128
                        g0 = int(gw0[t, b])
                        for k in range(nw_):
                            gw = g0 + k
                            ct = call_tiles[(b, gw // CALL_W)]
                            last = (b == 3 and k == nw_ - 1)
                            nc.tensor.matmul(
                                psZ[:], lhsT=ct[:, gw % CALL_W, :],
                                rhs=oh[:, wk, :], start=False, stop=last)
                            wk += 1
                    w = tail(t, psZ)
                    if w is not None:
                        writes[int(chunk_of[t])].append(w)
                return writes

            def tail1(t, psZ):
                Zs = zp.tile([128, 128], f32, tag="Zs")
                nc.scalar.activation(Zs[:], psZ[:], COPY)
                ps = pp.tile([128, HID], f32, tag="p1")
                nc.tensor.matmul(ps[:], lhsT=Zs[:], rhs=w1[:],
                                 start=True, stop=True)
                r1 = ep.tile([128, HID], f32, tag="r1")
                nc.vector.scalar_tensor_tensor(
                    r1[:], ps[:], dinv[:, t:t + 1], bb1[:], op0=MUL, op1=ADD)
                r1a = ep.tile([128, HID], f32, tag="r1a")
                nc.scalar.activation(r1a[:], r1[:],
                                     mybir.ActivationFunctionType.Relu)
                psT = pp.tile([HID, 128], f32, tag="pT")
                nc.tensor.transpose(psT[:], r1a[:], ident[:])
                r1T = ep.tile([HID, 128], f32, tag="r1T")
                nc.vector.tensor_copy(r1T[:], psT[:])
                ps2 = pp.tile([128, 128], f32, tag="p2")
                nc.tensor.matmul(ps2[:], lhsT=r1T[:], rhs=w2[:],
                                 start=True, stop=True)
                g2 = yresB[:, t * F:(t + 1) * F]
                nc.vector.tensor_scalar_mul(g2, ps2[:], dinv[:, t:t + 1])
                return nc.sync.dma_start(shard[t * 128:(t + 1) * 128, :], g2)

            def tail2(t, psZ):
                Zs = zp.tile([128, 128], f32, tag="Zs")
                nc.scalar.activation(Zs[:], psZ[:], COPY)
                psT2 = pp.tile([128, 128], f32, tag="p2")
                nc.tensor.transpose(psT2[:], Zs[:], ident[:])
                r2 = ep.tile([128, 128], f32, tag="r2")
                nc.vector.scalar_tensor_tensor(
                    r2[:], psT2[:], dinv[:, t:t + 1], bb2[:], op0=MUL, op1=ADD)
                y2 = yresA[:, t * F:(t + 1) * F]
                nc.scalar.activation(y2, r2[:],
                                     mybir.ActivationFunctionType.Relu,
                                     scale=dinv[:, t:t + 1])
                return nc.sync.dma_start(shard[t * 128:(t + 1) * 128, :], y2)

            def tail3(t, psZ):
                Zs = zp.tile([128, 128], f32, tag="Zs")
                nc.scalar.activation(Zs[:], psZ[:], COPY)
                ps = pp.tile([128, OUT_C], f32, tag="p1")
                nc.tensor.matmul(ps[:], lhsT=Zs[:], rhs=w3[:],
                                 start=True, stop=True)
                o3 = ep.tile([128, OUT_C], f32, tag="o3")
                nc.vector.scalar_tensor_tensor(
                    o3[:], ps[:], dinv[:, t:t + 1], bb3[:], op0=MUL, op1=ADD)
                nc.sync.dma_start(out_sh[t * 128:(t + 1) * 128, :], o3[:])
                return None

            with nc.named_scope("L1"):
                w1w = aggregate_layer(tabA0, yresA, tail1, ())
            with nc.named_scope("AG2"):
                cc2 = allgather(tabB, w1w)
            with nc.named_scope("L2"):
                w2w = aggregate_layer(tabB, yresB, tail2, cc2)
            with nc.named_scope("AG3"):
                cc3 = allgather(tabA, w2w)
            with nc.named_scope("L3"):
                aggregate_layer(tabA, yresA, tail3, cc3)

    nc.compile()
    return nc


# ----------------------------------------------------------------------------
# entry point
# ----------------------------------------------------------------------------
def kernel(x, edge_index, W1, b1, W2, b2, W3, b3, _trace=False):
    global LAST_RESULT
    from concourse.bass_utils import run_bass_kernel_spmd

    x = np.asarray(x, np.float32)
    edge_index = np.asarray(edge_index)

    gkey = ("prep", int(edge_index[:, ::997].sum()), edge_index.shape[1])
    if gkey not in _CACHE:
        _CACHE.clear()
        _CACHE[gkey] = _preprocess(edge_index)
    prep = _CACHE[gkey]
    core, local = prep["core"], prep["local"]

    if "prog" not in _CACHE:
        _CACHE["prog"] = _build_program(prep["geo"])
    nc = _CACHE["prog"]

    W1 = np.asarray(W1, np.float32)
    W2 = np.asarray(W2, np.float32)
    W3 = np.asarray(W3, np.float32)
    b1b = np.tile(np.asarray(b1, np.float32)[None, :], (128, 1))
    b2b = np.tile(np.asarray(b2, np.float32)[None, :], (128, 1))
    b3b = np.tile(np.asarray(b3, np.float32)[None, :], (128, 1))
    iota16 = np.tile(np.arange(128, dtype=np.float16)[None, :], (128, 1))
    ident16 = np.eye(128, dtype=np.float16)

    deg_n = prep["degT"]  # [C,128,NT]
    xps = []
    x_pres = []
    for c in range(NCORES):
        xc = np.zeros((USED, 128), np.float32)
        sel = core == c
        xc[local[sel]] = x[sel]
        dinv_n = 1.0 / np.sqrt(
            deg_n[c].transpose(1, 0).reshape(USED))      # per local node
        xp = (xc * dinv_n[:, None]).astype(np.float16)
        xps.append(xp)
        x_pres.append(np.ascontiguousarray(
            xp.reshape(NTILES, 128, 128).transpose(1, 0, 2).reshape(
                128, NTILES * F)))
    tab0 = np.ascontiguousarray(np.concatenate(xps, axis=0))
    in_maps = []
    for c in range(NCORES):
        in_maps.append(dict(
            x_pre=x_pres[c], tabA0=tab0,
            degT=np.ascontiguousarray(prep["degT"][c]),
            idxs=prep["idx"][c], dposT=prep["dposT"][c], iota16=iota16,
            ident16=ident16,
            W1=W1, W2=W2, W3=W3, b1b=b1b, b2b=b2b, b3b=b3b))

    res = run_bass_kernel_spmd(nc, in_maps, core_ids=list(range(NCORES)),
                               trace=_trace)
    LAST_RESULT = res

    out = np.empty((N, OUT_C), np.float32)
    for c in range(NCORES):
        sel = core == c
        out[sel] = res.results[c]["out_sh"][local[sel]]
    return out
